# revision 1
# baseline (speedup 1.0000x reference)
"""BetaTCVAE loss kernel for 8 TRN2 NeuronCores (Bass/Tile).

Math
----
reference:  out = (BETA-1)*tc + sum(kl)
  lp[i,j,d] = -0.5*((z_i - m_j)^2 * exp(-lv_j) + lv_j + LOG2PI)   (per dim d)
  log_qz_product[i] = sum_d logsumexp_j lp[i,j,d]
  log_qz[i]         = logsumexp_j sum_d lp[i,j,d]
  tc = mean_i(log_qz - log_qz_product)

Decomposition used here (per core, rows i sharded 256/core):
  * log_qz: S'[i,j] = sum_d(-0.5*w*z^2 + w*m*z - 0.5*(w*m^2+lv)) is a pair of
    [256x64]@[64x2048] matmuls plus a rank-1 term -> TensorEngine;
    log_qz[i] = logsumexp_j S'[i,j] - 32*LOG2PI.
  * log_qz_product: A[i,d] = sum_j q*exp(-0.5*w*(z-m)^2). With s=sqrt(w/2)
    the weight q = exp(-0.5*(lv+LOG2PI)) equals s/sqrt(pi), and
    exp(-0.5*w*(z-m)^2) = (sqrt(pi)/2)*DerivErf(s*z - s*m), so
      A_acc[i,d] = sum_j s * DerivErf(s*z - s*m) = 2*A[i,d].
    One ACT instruction per j-column batch (Derivative_Erf), one fused
    scalar_tensor_tensor accumulate per column on DVE/Pool.
  * Partition layout for the hot loop: p = (e,d), e = j-half, d = latent dim;
    free axis = all 256 local i. 1024 packed columns.
  * Final: out = (BETA-1)*(T_sum/B + K0) + KL_sum,
    K0 = -32*LOG2PI + 64*ln2  (host side, exact).
"""

import math
import sys

import numpy as np

if "/opt/trn_rl_repo" not in sys.path:
    sys.path.insert(0, "/opt/trn_rl_repo")

import concourse.bacc as bacc
import concourse.tile as tile
from concourse import mybir
from concourse.bass_utils import run_bass_kernel_spmd
from concourse.masks import make_identity

B, D, M = 2048, 64, 8
BL = B // M          # 256 local rows
NJT = B // 128       # 16 natural j-tiles
NCOL = B // 2        # 1024 packed columns (e-packing: j-halves on partitions)
KB = 8               # j-columns per DerivErf batch
F32 = mybir.dt.float32
BF16 = mybir.dt.bfloat16
LOG_2PI = math.log(2.0 * math.pi)
BETA = 6.0
K0 = -32.0 * LOG_2PI + 64.0 * math.log(2.0)

A = mybir.AluOpType
AF = mybir.ActivationFunctionType
AX = mybir.AxisListType


def _body(tc):
    nc = tc.nc
    kl_ext = nc.dram_tensor("kl", [BL, D], F32, kind="ExternalInput").ap()
    zm_ext = nc.dram_tensor("z_mean", [B, D], F32, kind="ExternalInput").ap()
    zlv_ext = nc.dram_tensor("z_logvar", [B, D], F32, kind="ExternalInput").ap()
    zs_ext = nc.dram_tensor("z_sampled", [BL, D], F32, kind="ExternalInput").ap()
    out_ext = nc.dram_tensor("out", [1, 2], F32, kind="ExternalOutput").ap()

    with (
        tc.tile_pool(name="cst", bufs=1) as cst,
        tc.tile_pool(name="mats", bufs=1) as mats,
        tc.tile_pool(name="ld", bufs=4) as ld,
        tc.tile_pool(name="yb", bufs=3) as yb,
        tc.tile_pool(name="db", bufs=3) as db,
    ):
        ident = cst.tile([128, 128], F32, tag="ident")
        make_identity(nc, ident)
        ones = cst.tile([128, 1], F32, tag="ones")
        nc.vector.memset(ones, 1.0)
        neghalf = cst.tile([128, 128], F32, tag="neghalf")
        nc.gpsimd.memset(neghalf, -0.5)

        # ---- load + transpose z_mean, z_logvar -> M_T/LV_T [64, 2048] ----
        m_t = mats.tile([64, B], F32, tag="m_t")
        lv_t = mats.tile([64, B], F32, tag="lv_t")
        z_t = mats.tile([64, BL], F32, tag="z_t")
        with tc.tile_pool(name="pst", bufs=4, space="PSUM") as pst:
            for t in range(NJT):
                nat = ld.tile([128, D], F32, tag="nat")
                nc.sync.dma_start(out=nat, in_=zm_ext[t * 128:(t + 1) * 128, :])
                ps = pst.tile([64, 128], F32, tag="tp")
                nc.tensor.transpose(ps, nat, ident)
                nc.vector.tensor_copy(out=m_t[0:64, t * 128:(t + 1) * 128], in_=ps)
            for t in range(NJT):
                nat = ld.tile([128, D], F32, tag="nat")
                nc.sync.dma_start(out=nat, in_=zlv_ext[t * 128:(t + 1) * 128, :])
                ps = pst.tile([64, 128], F32, tag="tp")
                nc.tensor.transpose(ps, nat, ident)
                nc.vector.tensor_copy(out=lv_t[0:64, t * 128:(t + 1) * 128], in_=ps)
            for t in range(2):
                nat = ld.tile([128, D], F32, tag="nat")
                nc.sync.dma_start(out=nat, in_=zs_ext[t * 128:(t + 1) * 128, :])
                ps = pst.tile([64, 128], F32, tag="tp")
                nc.tensor.transpose(ps, nat, ident)
                nc.vector.tensor_copy(out=z_t[0:64, t * 128:(t + 1) * 128], in_=ps)

        # ---- kl partial sum ----
        ks2 = mats.tile([128, 2], F32, tag="ks2")
        for t in range(2):
            klt = ld.tile([128, D], F32, tag="klt", bufs=2)
            nc.sync.dma_start(out=klt, in_=kl_ext[t * 128:(t + 1) * 128, :])
            nc.vector.tensor_reduce(out=ks2[:, t:t + 1], in_=klt, axis=AX.X, op=A.add)
        kss = mats.tile([128, 1], F32, tag="kss")
        nc.vector.tensor_reduce(out=kss, in_=ks2, axis=AX.X, op=A.add)

        # ---- prep params (T-layout, [64, 2048]) ----
        s_t = mats.tile([64, B], F32, tag="s_t")
        #  s = exp(-lv/2)/sqrt(2) = sqrt(w/2)
        bias_l2 = cst.tile([128, 1], F32, tag="bias_l2")
        nc.gpsimd.memset(bias_l2, -0.5 * math.log(2.0))
        nc.scalar.activation(out=s_t[0:64, :], in_=lv_t[0:64, :], func=AF.Exp,
                             bias=bias_l2[0:64, :], scale=-0.5)
        w_t = mats.tile([64, B], F32, tag="w_t")
        nc.vector.scalar_tensor_tensor(out=w_t[0:64, :], in0=s_t[0:64, :],
                                       scalar=2.0, in1=s_t[0:64, :],
                                       op0=A.mult, op1=A.mult)
        wm_t = mats.tile([64, B], F32, tag="wm_t")
        nc.vector.tensor_mul(out=wm_t[0:64, :], in0=w_t[0:64, :],
                             in1=m_t[0:64, :])
        t3 = mats.tile([64, B], F32, tag="t3")
        nc.gpsimd.tensor_mul(out=t3[0:64, :], in0=wm_t[0:64, :], in1=m_t[0:64, :])
        nc.gpsimd.tensor_add(out=t3[0:64, :], in0=t3[0:64, :], in1=lv_t[0:64, :])

        z2n_t = mats.tile([64, BL], F32, tag="z2n_t")
        nc.scalar.activation(out=z2n_t[0:64, :], in_=z_t[0:64, :], func=AF.Square,
                             bias=0.0, scale=1.0)
        nc.vector.tensor_scalar(out=z2n_t[0:64, :], in0=z2n_t[0:64, :],
                                scalar1=-0.5, scalar2=None, op0=A.mult)

        # ---- replicated bf16 tiles for the hot loop (partition = (h,d)) ----
        m_rep = mats.tile([128, B], BF16, tag="m_rep")
        nc.vector.tensor_copy(out=m_rep[0:64, :], in_=m_t[0:64, :])
        nc.sync.dma_start(out=m_rep[64:128, :], in_=m_rep[0:64, :])
        s_rep = mats.tile([128, B], BF16, tag="s_rep")
        nc.vector.tensor_copy(out=s_rep[0:64, :], in_=s_t[0:64, :])
        nc.sync.dma_start(out=s_rep[64:128, :], in_=s_rep[0:64, :])
        # z columns: partition p=(h,d) holds z[i = g + 128h, d] at column g
        zpk = mats.tile([128, 128], F32, tag="zpk")
        nc.sync.dma_start(out=zpk[0:64, :], in_=z_t[0:64, 0:128])
        nc.sync.dma_start(out=zpk[64:128, :], in_=z_t[0:64, 128:256])
        nzpk = mats.tile([128, 128], F32, tag="nzpk")
        nc.vector.tensor_scalar(out=nzpk, in0=zpk, scalar1=-1.0, scalar2=None,
                                op0=A.mult)

        # A[p=(h,d), g] = sum_j s * DerivErf(s*(z-m)) per (i=g+128h, d)
        a_mat = mats.tile([128, 128], F32, tag="a_mat")

        # ---- HOT LOOP: one group per z-column (i), j = full 2048 free ----
        # u = m - z_g  (sign-free under DerivErf), y = u*s, D = DerivErf(y),
        # A[:, g] = sum_j s*D  (affine_mul_reduce on DVE).
        NG = 128
        with (
            tc.tile_pool(name="ut", bufs=3) as up,
            tc.tile_pool(name="yt", bufs=3) as yp,
            tc.tile_pool(name="dt", bufs=3) as dp,
            tc.tile_pool(name="et", bufs=2) as ep,
        ):
            for g in range(NG):
                u_t = up.tile([128, B], BF16, tag="u")
                if g % 2 == 0:
                    nc.vector.tensor_scalar(out=u_t, in0=m_rep,
                                            scalar1=zpk[:, g:g + 1],
                                            scalar2=None, op0=A.subtract)
                else:
                    nc.scalar.activation(out=u_t, in_=m_rep, func=AF.Identity,
                                         bias=nzpk[:, g:g + 1], scale=1.0)
                y_t = yp.tile([128, B], BF16, tag="y")
                yeng = nc.vector if (g % 6 == 5) else nc.gpsimd
                yeng.tensor_tensor(out=y_t, in0=u_t, in1=s_rep, op=A.mult)
                d_t = dp.tile([128, B], BF16, tag="d")
                nc.scalar.activation(out=d_t, in_=y_t, func=AF.Derivative_Erf,
                                     bias=0.0, scale=1.0)
                e_t = ep.tile([128, B], BF16, tag="e")
                nc.vector.affine_mul_reduce(out=e_t,
                                            accum_out=a_mat[:, g:g + 1],
                                            in0=d_t, in1=s_rep,
                                            scale=1.0, bias=0.0)

        # ---- A epilogue: log then partition-reduce over d (per h-half) ----
        ln_a = mats.tile([128, 128], F32, tag="ln_a")
        nc.scalar.activation(out=ln_a, in_=a_mat, func=AF.Ln,
                             bias=0.0, scale=1.0)

        # ---- S' matmuls + logsumexp epilogue ----
        contrib = []
        with (
            tc.tile_pool(name="psp", bufs=1, space="PSUM") as psp,
            tc.tile_pool(name="psm", bufs=2, space="PSUM") as psm,
            tc.tile_pool(name="scr", bufs=2) as scr,
        ):
            for it in range(2):
                isl = slice(it * 128, (it + 1) * 128)
                sps = []
                for jb in range(4):
                    jsl = slice(jb * 512, (jb + 1) * 512)
                    sp = psp.tile([128, 512], F32, tag=f"sp{jb}")
                    nc.tensor.matmul(sp, lhsT=z2n_t[0:64, isl], rhs=w_t[0:64, jsl],
                                     start=True, stop=False)
                    nc.tensor.matmul(sp, lhsT=z_t[0:64, isl], rhs=wm_t[0:64, jsl],
                                     start=False, stop=False)
                    nc.tensor.matmul(sp, lhsT=neghalf[0:64, :], rhs=t3[0:64, jsl],
                                     start=False, stop=True)
                    sps.append(sp)
                mx4 = mats.tile([128, 4], F32, tag="mx4", bufs=2)
                for jb in range(4):
                    nc.vector.tensor_reduce(out=mx4[:, jb:jb + 1], in_=sps[jb],
                                            axis=AX.X, op=A.max)
                nmx = mats.tile([128, 1], F32, tag="nmx", bufs=2)
                nc.vector.tensor_reduce(out=nmx, in_=mx4, axis=AX.X, op=A.max,
                                        negate=True)
                es4 = mats.tile([128, 4], F32, tag="es4", bufs=2)
                for jb in range(4):
                    sc = scr.tile([128, 512], F32, tag="sc")
                    nc.scalar.activation(out=sc, in_=sps[jb], func=AF.Exp,
                                         bias=nmx, scale=1.0,
                                         accum_out=es4[:, jb:jb + 1])
                esum = mats.tile([128, 1], F32, tag="esum", bufs=2)
                nc.vector.tensor_reduce(out=esum, in_=es4, axis=AX.X, op=A.add)
                lqz = mats.tile([128, 1], F32, tag="lqz", bufs=2)
                nc.scalar.activation(out=lqz, in_=esum, func=AF.Ln,
                                     bias=0.0, scale=1.0)
                # lqz - P  (P via ones-matmul over d), both [128,1]
                # i-tile 0 <-> h=0 lives on partitions 0:64, i-tile 1 on 64:128
                psl = slice(it * 64, (it + 1) * 64)
                pps = psm.tile([128, 1], F32, tag="pp")
                nc.tensor.matmul(pps, lhsT=ln_a[psl, :], rhs=ones[psl, :],
                                 start=True, stop=True)
                ctr = mats.tile([128, 1], F32, tag="ctr", bufs=2)
                # ctr = (lqz + (-1)*mx4_max...) careful: lqz currently ln(esum);
                # full log_qz = lqz + mx ; contrib = lqz + mx - P
                mx = mats.tile([128, 1], F32, tag="mx", bufs=2)
                nc.vector.tensor_scalar(out=mx, in0=nmx, scalar1=-1.0,
                                        scalar2=None, op0=A.mult)
                nc.vector.tensor_add(out=lqz, in0=lqz, in1=mx)
                nc.vector.tensor_sub(out=ctr, in0=lqz, in1=pps)
                contrib.append(ctr)

            # ---- final scalars ----
            fps = psm.tile([1, 2], F32, tag="fps")
            nc.tensor.matmul(fps[0:1, 0:1], lhsT=contrib[0], rhs=ones,
                             start=True, stop=False)
            nc.tensor.matmul(fps[0:1, 0:1], lhsT=contrib[1], rhs=ones,
                             start=False, stop=True)
            nc.tensor.matmul(fps[0:1, 1:2], lhsT=kss, rhs=ones,
                             start=True, stop=True)
            out_sb = mats.tile([1, 2], F32, tag="out_sb")
            nc.vector.tensor_copy(out=out_sb[0:1, :], in_=fps[0:1, :])
            nc.sync.dma_start(out=out_ext, in_=out_sb[0:1, :])


_NC_CACHE = {}


def _get_nc():
    if "nc" not in _NC_CACHE:
        nc = bacc.Bacc("TRN2", target_bir_lowering=False, debug=False,
                       num_devices=M)
        with tile.TileContext(nc) as tc:
            _body(tc)
        nc.compile()
        _NC_CACHE["nc"] = nc
    return _NC_CACHE["nc"]


def kernel(kl, z_mean, z_logvar, z_sampled, _trace=False, _tmpdir=None):
    kl = np.ascontiguousarray(kl, dtype=np.float32)
    z_mean = np.ascontiguousarray(z_mean, dtype=np.float32)
    z_logvar = np.ascontiguousarray(z_logvar, dtype=np.float32)
    z_sampled = np.ascontiguousarray(z_sampled, dtype=np.float32)
    nc = _get_nc()
    in_maps = []
    for c in range(M):
        sl = slice(c * BL, (c + 1) * BL)
        in_maps.append({
            "kl": np.ascontiguousarray(kl[sl]),
            "z_mean": z_mean,
            "z_logvar": z_logvar,
            "z_sampled": np.ascontiguousarray(z_sampled[sl]),
        })
    res = run_bass_kernel_spmd(nc, in_maps, list(range(M)), trace=_trace,
                               tmpdir=_tmpdir)
    t_sum = 0.0
    kl_sum = 0.0
    for c in range(M):
        o = res.results[c]["out"]
        t_sum += float(o[0, 0])
        kl_sum += float(o[0, 1])
    val = (BETA - 1.0) * (t_sum / B + K0) + kl_sum
    out = np.float32(val)
    if _trace:
        return out, res
    return out



# revision 4
# speedup vs baseline: 2.0203x; 2.0203x over previous
"""BetaTCVAE loss kernel for 8 TRN2 NeuronCores (Bass/Tile).

Math
----
reference:  out = (BETA-1)*tc + sum(kl)
  lp[i,j,d] = -0.5*((z_i - m_j)^2 * exp(-lv_j) + lv_j + LOG2PI)   (per dim d)
  log_qz_product[i] = sum_d logsumexp_j lp[i,j,d]
  log_qz[i]         = logsumexp_j sum_d lp[i,j,d]
  tc = mean_i(log_qz - log_qz_product)

Decomposition used here (per core, rows i sharded 256/core):
  With s^2 = exp(-lv)/2 define the shifted exponent
      t[i,j,d] = -s2*z^2 + wm*z + e,
      wm = 2*s2*m,  e = -0.5*(wm*m + lv) - 0.5*ln2
  so that t = lp + (LOG2PI - ln2)/2 per dim.  Then
      A[i,d]   = sum_j exp(t[i,j,d])       ->  P[i] = sum_d ln A[i,d]
      S[i,j]   = sum_d t[i,j,d]            ->  lqz[i] = logsumexp_j S[i,j]
      contrib[i] = lqz[i] - P[i]           (shift constants cancel)
  tc = mean_i contrib.

Mapping to engines:
  * t for one latent dim d is a rank-3 bilinear form: one [3 x 128] lhsT
    (-z^2, z, 1 per row-i) against a [3 x 512] rhs (s2, wm, e per col-j)
    -> 4 matmuls fill a [128 x 2048] PSUM tile on the Tensor engine.
  * A[i,d] = one Activation instruction: Exp over the PSUM tile with
    accum_out giving the j-sum directly.  ACT is the only engine doing a
    full pass over the [256 x 2048 x 64] log-density volume.
  * S via three 64-contraction bf16 matmuls (same H rows), classic
    max-shifted logsumexp epilogue.
  * Final: out = (BETA-1)*(T_sum/B) + KL_sum  (host side).
"""

import math
import sys

import numpy as np

if "/opt/trn_rl_repo" not in sys.path:
    sys.path.insert(0, "/opt/trn_rl_repo")

import concourse.bacc as bacc
import concourse.tile as tile
from concourse import mybir
from concourse.bass_utils import run_bass_kernel_spmd
from concourse.masks import make_identity

B, D, M = 2048, 64, 8
BL = B // M          # 256 local rows
NJT = B // 128       # 16 natural j-tiles
DCH = 8              # latent dims per H chunk
NCH = D // DCH       # 8 chunks
F32 = mybir.dt.float32
BF16 = mybir.dt.bfloat16
LN2 = math.log(2.0)
BETA = 6.0

A = mybir.AluOpType
AF = mybir.ActivationFunctionType
AX = mybir.AxisListType


def _body(tc):
    nc = tc.nc
    kl_ext = nc.dram_tensor("kl", [BL, D], F32, kind="ExternalInput").ap()
    zm_ext = nc.dram_tensor("z_mean", [B, D], F32, kind="ExternalInput").ap()
    zlv_ext = nc.dram_tensor("z_logvar", [B, D], F32, kind="ExternalInput").ap()
    zs_ext = nc.dram_tensor("z_sampled", [BL, D], F32, kind="ExternalInput").ap()
    out_ext = nc.dram_tensor("out", [1, 2], F32, kind="ExternalOutput").ap()

    with (
        tc.tile_pool(name="cst", bufs=1) as cst,
        tc.tile_pool(name="mats", bufs=1) as mats,
        tc.tile_pool(name="ld", bufs=4) as ld,
    ):
        ident = cst.tile([128, 128], F32, tag="ident")
        make_identity(nc, ident)
        ones = cst.tile([128, 1], F32, tag="ones")
        nc.vector.memset(ones, 1.0)

        # ---- load + transpose z_mean, z_logvar -> [64, 2048] f32 ----
        m_t = mats.tile([64, B], F32, tag="m_t")
        lv_t = mats.tile([64, B], F32, tag="lv_t")
        z_t = mats.tile([64, BL], F32, tag="z_t")
        with tc.tile_pool(name="pst", bufs=4, space="PSUM") as pst:
            for t in range(NJT):
                nat = ld.tile([128, D], F32, tag="nat")
                nc.sync.dma_start(out=nat, in_=zm_ext[t * 128:(t + 1) * 128, :])
                ps = pst.tile([64, 128], F32, tag="tp")
                nc.tensor.transpose(ps, nat, ident)
                nc.vector.tensor_copy(out=m_t[0:64, t * 128:(t + 1) * 128], in_=ps)
            for t in range(NJT):
                nat = ld.tile([128, D], F32, tag="nat")
                nc.sync.dma_start(out=nat, in_=zlv_ext[t * 128:(t + 1) * 128, :])
                ps = pst.tile([64, 128], F32, tag="tp")
                nc.tensor.transpose(ps, nat, ident)
                nc.vector.tensor_copy(out=lv_t[0:64, t * 128:(t + 1) * 128], in_=ps)
            for t in range(2):
                nat = ld.tile([128, D], F32, tag="nat")
                nc.sync.dma_start(out=nat, in_=zs_ext[t * 128:(t + 1) * 128, :])
                ps = pst.tile([64, 128], F32, tag="tp")
                nc.tensor.transpose(ps, nat, ident)
                nc.vector.tensor_copy(out=z_t[0:64, t * 128:(t + 1) * 128], in_=ps)

        # ---- kl partial sum ----
        ks2 = mats.tile([128, 2], F32, tag="ks2")
        for t in range(2):
            klt = ld.tile([128, D], F32, tag="klt", bufs=2)
            nc.sync.dma_start(out=klt, in_=kl_ext[t * 128:(t + 1) * 128, :])
            nc.vector.tensor_reduce(out=ks2[:, t:t + 1], in_=klt, axis=AX.X, op=A.add)
        kss = mats.tile([128, 1], F32, tag="kss")
        nc.vector.tensor_reduce(out=kss, in_=ks2, axis=AX.X, op=A.add)

        # ---- prep H rows (j side, [64 d, 2048 j]) ----
        # s2 = exp(-lv)/2; wm = 2*s2*m; e = -0.5*(wm*m + lv) - ln2/2
        s2_t = mats.tile([64, B], F32, tag="s2_t")
        bias_l2 = cst.tile([128, 1], F32, tag="bias_l2")
        nc.gpsimd.memset(bias_l2, math.log(0.5))
        nc.scalar.activation(out=s2_t[0:64, :], in_=lv_t[0:64, :], func=AF.Exp,
                             bias=bias_l2[0:64, :], scale=-1.0)
        hrow0 = mats.tile([64, B], BF16, tag="hrow0")
        nc.gpsimd.tensor_copy(out=hrow0[0:64, :], in_=s2_t[0:64, :])
        wm_t = mats.tile([64, B], F32, tag="wm_t")
        nc.vector.scalar_tensor_tensor(out=wm_t[0:64, :], in0=s2_t[0:64, :],
                                       scalar=2.0, in1=m_t[0:64, :],
                                       op0=A.mult, op1=A.mult)
        hrow1 = mats.tile([64, B], BF16, tag="hrow1")
        nc.gpsimd.tensor_copy(out=hrow1[0:64, :], in_=wm_t[0:64, :])
        # e1 = (-0.5*wm)*m ; lvh = -0.5*lv - ln2/2 ; e = e1 + lvh
        e1_t = mats.tile([64, B], F32, tag="e1_t")
        nc.vector.scalar_tensor_tensor(out=e1_t[0:64, :], in0=wm_t[0:64, :],
                                       scalar=-0.5, in1=m_t[0:64, :],
                                       op0=A.mult, op1=A.mult)
        lvh_t = mats.tile([64, B], F32, tag="lvh_t")
        nc.vector.tensor_scalar(out=lvh_t[0:64, :], in0=lv_t[0:64, :],
                                scalar1=-0.5, scalar2=-0.5 * LN2,
                                op0=A.mult, op1=A.add)
        hrow2 = mats.tile([64, B], BF16, tag="hrow2")
        nc.vector.tensor_add(out=hrow2[0:64, :], in0=e1_t[0:64, :],
                             in1=lvh_t[0:64, :])

        # ---- prep G rows (i side) ----
        z2f = mats.tile([64, BL], F32, tag="z2f")
        nc.vector.tensor_mul(out=z2f[0:64, :], in0=z_t[0:64, :], in1=z_t[0:64, :])
        nz2_b = mats.tile([64, BL], BF16, tag="nz2_b")
        nc.vector.tensor_scalar(out=nz2_b[0:64, :], in0=z2f[0:64, :],
                                scalar1=-1.0, scalar2=None, op0=A.mult)
        z_b = mats.tile([64, BL], BF16, tag="z_b")
        nc.vector.tensor_copy(out=z_b[0:64, :], in_=z_t[0:64, :])
        ones_b = mats.tile([64, BL], BF16, tag="ones_b")
        nc.gpsimd.memset(ones_b, 1.0)

        # G_mega [3, 64*256]: per-(d,it) lhsT slices, d-major from [64,256] rows
        g_mega = mats.tile([3, D * BL], BF16, tag="g_mega")
        nc.sync.dma_start(out=g_mega[0:1, :], in_=nz2_b[0:64, :])
        nc.sync.dma_start(out=g_mega[1:2, :], in_=z_b[0:64, :])
        nc.sync.dma_start(out=g_mega[2:3, :], in_=ones_b[0:64, :])

        # A[i,d] accumulators, one per i-tile
        a_mat0 = mats.tile([128, D], F32, tag="a_mat0")
        a_mat1 = mats.tile([128, D], F32, tag="a_mat1")
        a_mat = [a_mat0, a_mat1]

        # ---- A hot loop: PE bilinear form -> ACT Exp + accum ----
        with (
            tc.tile_pool(name="hp", bufs=2) as hp,
            tc.tile_pool(name="pa", bufs=2, space="PSUM") as pa,
            tc.tile_pool(name="ep", bufs=2) as ep,
        ):
            for c in range(NCH):
                hch = hp.tile([3, DCH * B], BF16, tag="hch")
                dsl = slice(c * DCH, (c + 1) * DCH)
                nc.sync.dma_start(out=hch[0:1, :], in_=hrow0[dsl, :])
                nc.sync.dma_start(out=hch[1:2, :], in_=hrow1[dsl, :])
                nc.sync.dma_start(out=hch[2:3, :], in_=hrow2[dsl, :])
                for dd in range(DCH):
                    d = c * DCH + dd
                    for it in range(2):
                        tp = pa.tile([128, B], F32, tag="tp")
                        lhs = g_mega[0:3, d * BL + it * 128: d * BL + (it + 1) * 128]
                        for jb in range(4):
                            nc.tensor.matmul(
                                tp[:, jb * 512:(jb + 1) * 512], lhsT=lhs,
                                rhs=hch[0:3, dd * B + jb * 512: dd * B + (jb + 1) * 512],
                                start=True, stop=True)
                        e_t = ep.tile([128, B], BF16, tag="e")
                        nc.scalar.activation(out=e_t, in_=tp, func=AF.Exp,
                                             bias=0.0, scale=1.0,
                                             accum_out=a_mat[it][:, d:d + 1])

        # ---- S matmuls + logsumexp (pre-Ln part) ----
        nmxs, esums = [], []
        with (
            tc.tile_pool(name="psp", bufs=1, space="PSUM") as psp,
            tc.tile_pool(name="scr", bufs=2) as scr,
        ):
            for it in range(2):
                isl = slice(it * 128, (it + 1) * 128)
                sps = []
                for jb in range(4):
                    jsl = slice(jb * 512, (jb + 1) * 512)
                    sp = psp.tile([128, 512], F32, tag=f"sp{jb}")
                    nc.tensor.matmul(sp, lhsT=nz2_b[0:64, isl], rhs=hrow0[0:64, jsl],
                                     start=True, stop=False)
                    nc.tensor.matmul(sp, lhsT=z_b[0:64, isl], rhs=hrow1[0:64, jsl],
                                     start=False, stop=False)
                    nc.tensor.matmul(sp, lhsT=ones_b[0:64, 0:128], rhs=hrow2[0:64, jsl],
                                     start=False, stop=True)
                    sps.append(sp)
                mx4 = mats.tile([128, 4], F32, tag="mx4", bufs=2)
                for jb in range(4):
                    nc.vector.tensor_reduce(out=mx4[:, jb:jb + 1], in_=sps[jb],
                                            axis=AX.X, op=A.max)
                nmx = mats.tile([128, 1], F32, tag="nmx", bufs=2)
                nc.vector.tensor_reduce(out=nmx, in_=mx4, axis=AX.X, op=A.max,
                                        negate=True)
                es4 = mats.tile([128, 4], F32, tag="es4", bufs=2)
                for jb in range(4):
                    sc = scr.tile([128, 512], BF16, tag="sc")
                    nc.scalar.activation(out=sc, in_=sps[jb], func=AF.Exp,
                                         bias=nmx, scale=1.0,
                                         accum_out=es4[:, jb:jb + 1])
                esum = mats.tile([128, 1], F32, tag="esum", bufs=2)
                nc.vector.tensor_reduce(out=esum, in_=es4, axis=AX.X, op=A.add)
                nmxs.append(nmx)
                esums.append(esum)

        # ---- Ln epilogue + final scalars ----
        with tc.tile_pool(name="psm", bufs=2, space="PSUM") as psm:
            contrib = []
            for it in range(2):
                ln_a = mats.tile([128, D], F32, tag="ln_a", bufs=2)
                nc.scalar.activation(out=ln_a, in_=a_mat[it], func=AF.Ln,
                                     bias=0.0, scale=1.0)
                p_col = mats.tile([128, 1], F32, tag="p_col", bufs=2)
                nc.vector.tensor_reduce(out=p_col, in_=ln_a, axis=AX.X, op=A.add)
                lqz = mats.tile([128, 1], F32, tag="lqz", bufs=2)
                nc.scalar.activation(out=lqz, in_=esums[it], func=AF.Ln,
                                     bias=0.0, scale=1.0)
                mx = mats.tile([128, 1], F32, tag="mx", bufs=2)
                nc.vector.tensor_scalar(out=mx, in0=nmxs[it], scalar1=-1.0,
                                        scalar2=None, op0=A.mult)
                nc.vector.tensor_add(out=lqz, in0=lqz, in1=mx)
                ctr = mats.tile([128, 1], F32, tag="ctr", bufs=2)
                nc.vector.tensor_sub(out=ctr, in0=lqz, in1=p_col)
                contrib.append(ctr)

            fps = psm.tile([1, 2], F32, tag="fps")
            nc.tensor.matmul(fps[0:1, 0:1], lhsT=contrib[0], rhs=ones,
                             start=True, stop=False)
            nc.tensor.matmul(fps[0:1, 0:1], lhsT=contrib[1], rhs=ones,
                             start=False, stop=True)
            nc.tensor.matmul(fps[0:1, 1:2], lhsT=kss, rhs=ones,
                             start=True, stop=True)
            out_sb = mats.tile([1, 2], F32, tag="out_sb")
            nc.vector.tensor_copy(out=out_sb[0:1, :], in_=fps[0:1, :])
            nc.sync.dma_start(out=out_ext, in_=out_sb[0:1, :])


_NC_CACHE = {}


def _get_nc():
    if "nc" not in _NC_CACHE:
        nc = bacc.Bacc("TRN2", target_bir_lowering=False, debug=False,
                       num_devices=M)
        with tile.TileContext(nc) as tc:
            _body(tc)
        nc.compile()
        _NC_CACHE["nc"] = nc
    return _NC_CACHE["nc"]


def kernel(kl, z_mean, z_logvar, z_sampled, _trace=False, _tmpdir=None):
    kl = np.ascontiguousarray(kl, dtype=np.float32)
    z_mean = np.ascontiguousarray(z_mean, dtype=np.float32)
    z_logvar = np.ascontiguousarray(z_logvar, dtype=np.float32)
    z_sampled = np.ascontiguousarray(z_sampled, dtype=np.float32)
    nc = _get_nc()
    in_maps = []
    for c in range(M):
        sl = slice(c * BL, (c + 1) * BL)
        in_maps.append({
            "kl": np.ascontiguousarray(kl[sl]),
            "z_mean": z_mean,
            "z_logvar": z_logvar,
            "z_sampled": np.ascontiguousarray(z_sampled[sl]),
        })
    res = run_bass_kernel_spmd(nc, in_maps, list(range(M)), trace=_trace,
                               tmpdir=_tmpdir)
    t_sum = 0.0
    kl_sum = 0.0
    for c in range(M):
        o = res.results[c]["out"]
        t_sum += float(o[0, 0])
        kl_sum += float(o[0, 1])
    val = (BETA - 1.0) * (t_sum / B) + kl_sum
    out = np.float32(val)
    if _trace:
        return out, res
    return out


# revision 6
# speedup vs baseline: 2.3591x; 1.1677x over previous
"""BetaTCVAE loss kernel for 8 TRN2 NeuronCores (Bass/Tile).

Math
----
reference:  out = (BETA-1)*tc + sum(kl)
  lp[i,j,d] = -0.5*((z_i - m_j)^2 * exp(-lv_j) + lv_j + LOG2PI)   (per dim d)
  log_qz_product[i] = sum_d logsumexp_j lp[i,j,d]
  log_qz[i]         = logsumexp_j sum_d lp[i,j,d]
  tc = mean_i(log_qz - log_qz_product)

Decomposition used here (per core, rows i sharded 256/core):
  With s^2 = exp(-lv)/2 define the shifted exponent
      t[i,j,d] = -s2*z^2 + wm*z + e,
      wm = 2*s2*m,  e = -0.5*(wm*m + lv) - 0.5*ln2
  so that t = lp + (LOG2PI - ln2)/2 per dim.  Then
      A[i,d]   = sum_j exp(t[i,j,d])       ->  P[i] = sum_d ln A[i,d]
      S[i,j]   = sum_d t[i,j,d]            ->  lqz[i] = logsumexp_j S[i,j]
      contrib[i] = lqz[i] - P[i]           (shift constants cancel)
  tc = mean_i contrib.

Mapping to engines:
  * t for one latent dim d is a rank-3 bilinear form: one [3 x 128] lhsT
    (-z^2, z, 1 per row-i) against a [3 x 512] rhs (s2, wm, e per col-j)
    -> 4 matmuls fill a [128 x 2048] PSUM tile on the Tensor engine.
  * A[i,d] = one Activation instruction: Exp over the PSUM tile with
    accum_out giving the j-sum directly.  ACT is the only engine doing a
    full pass over the [256 x 2048 x 64] log-density volume.
  * S via three 64-contraction bf16 matmuls (same H rows), classic
    max-shifted logsumexp epilogue.
  * Final: out = (BETA-1)*(T_sum/B) + KL_sum  (host side).
"""

import math
import sys

import numpy as np

if "/opt/trn_rl_repo" not in sys.path:
    sys.path.insert(0, "/opt/trn_rl_repo")

import concourse.bacc as bacc
import concourse.tile as tile
from concourse import mybir
from concourse.bass_utils import run_bass_kernel_spmd
from concourse.masks import make_identity

B, D, M = 2048, 64, 8
BL = B // M          # 256 local rows
NJT = B // 128       # 16 natural j-tiles
DCH = 8              # latent dims per H chunk
NCH = D // DCH       # 8 chunks
F32 = mybir.dt.float32
BF16 = mybir.dt.bfloat16
LN2 = math.log(2.0)
BETA = 6.0

A = mybir.AluOpType
AF = mybir.ActivationFunctionType
AX = mybir.AxisListType


def _body(tc):
    nc = tc.nc
    kl_ext = nc.dram_tensor("kl", [BL, D], F32, kind="ExternalInput").ap()
    zm_ext = nc.dram_tensor("z_mean", [B, D], F32, kind="ExternalInput").ap()
    zlv_ext = nc.dram_tensor("z_logvar", [B, D], F32, kind="ExternalInput").ap()
    zs_ext = nc.dram_tensor("z_sampled", [BL, D], F32, kind="ExternalInput").ap()
    out_ext = nc.dram_tensor("out", [1, 2], F32, kind="ExternalOutput").ap()

    with (
        tc.tile_pool(name="cst", bufs=1) as cst,
        tc.tile_pool(name="mats", bufs=1) as mats,
        tc.tile_pool(name="ld", bufs=4) as ld,
    ):
        ident = cst.tile([128, 128], F32, tag="ident")
        make_identity(nc, ident)
        ones = cst.tile([128, 1], F32, tag="ones")
        nc.vector.memset(ones, 1.0)

        # ---- load + transpose z_mean, z_logvar -> [64, 2048] f32 ----
        m_t = mats.tile([64, B], F32, tag="m_t")
        lv_t = mats.tile([64, B], F32, tag="lv_t")
        z_t = mats.tile([64, BL], F32, tag="z_t")
        with tc.tile_pool(name="pst", bufs=4, space="PSUM") as pst:
            for t in range(NJT):
                nat = ld.tile([128, D], F32, tag="nat")
                nc.sync.dma_start(out=nat, in_=zm_ext[t * 128:(t + 1) * 128, :])
                ps = pst.tile([64, 128], F32, tag="tp")
                nc.tensor.transpose(ps, nat, ident)
                nc.vector.tensor_copy(out=m_t[0:64, t * 128:(t + 1) * 128], in_=ps)
            for t in range(NJT):
                nat = ld.tile([128, D], F32, tag="nat")
                nc.sync.dma_start(out=nat, in_=zlv_ext[t * 128:(t + 1) * 128, :])
                ps = pst.tile([64, 128], F32, tag="tp")
                nc.tensor.transpose(ps, nat, ident)
                nc.vector.tensor_copy(out=lv_t[0:64, t * 128:(t + 1) * 128], in_=ps)
            for t in range(2):
                nat = ld.tile([128, D], F32, tag="nat")
                nc.sync.dma_start(out=nat, in_=zs_ext[t * 128:(t + 1) * 128, :])
                ps = pst.tile([64, 128], F32, tag="tp")
                nc.tensor.transpose(ps, nat, ident)
                nc.vector.tensor_copy(out=z_t[0:64, t * 128:(t + 1) * 128], in_=ps)

        # ---- kl partial sum ----
        ks2 = mats.tile([128, 2], F32, tag="ks2")
        for t in range(2):
            klt = ld.tile([128, D], F32, tag="klt", bufs=2)
            nc.sync.dma_start(out=klt, in_=kl_ext[t * 128:(t + 1) * 128, :])
            nc.vector.tensor_reduce(out=ks2[:, t:t + 1], in_=klt, axis=AX.X, op=A.add)
        kss = mats.tile([128, 1], F32, tag="kss")
        nc.vector.tensor_reduce(out=kss, in_=ks2, axis=AX.X, op=A.add)

        # ---- prep H rows (j side, [64 d, 2048 j]) ----
        # s2 = exp(-lv)/2; wm = 2*s2*m; e = -0.5*(wm*m + lv) - ln2/2
        bias_l2 = cst.tile([128, 1], F32, tag="bias_l2")
        nc.gpsimd.memset(bias_l2, math.log(0.5))
        hrow0 = mats.tile([64, B], BF16, tag="hrow0")
        nc.scalar.activation(out=hrow0[0:64, :], in_=lv_t[0:64, :], func=AF.Exp,
                             bias=bias_l2[0:64, :], scale=-1.0)
        wm_t = mats.tile([64, B], F32, tag="wm_t")
        nc.vector.scalar_tensor_tensor(out=wm_t[0:64, :], in0=hrow0[0:64, :],
                                       scalar=2.0, in1=m_t[0:64, :],
                                       op0=A.mult, op1=A.mult)
        hrow1 = mats.tile([64, B], BF16, tag="hrow1")
        nc.vector.tensor_copy(out=hrow1[0:64, :], in_=wm_t[0:64, :])
        # e1 = (-0.5*wm)*m ; lvh = -0.5*lv - ln2/2 ; e = e1 + lvh
        e1_t = mats.tile([64, B], F32, tag="e1_t")
        nc.vector.scalar_tensor_tensor(out=e1_t[0:64, :], in0=wm_t[0:64, :],
                                       scalar=-0.5, in1=m_t[0:64, :],
                                       op0=A.mult, op1=A.mult)
        lvh_t = mats.tile([64, B], F32, tag="lvh_t")
        nc.vector.tensor_scalar(out=lvh_t[0:64, :], in0=lv_t[0:64, :],
                                scalar1=-0.5, scalar2=-0.5 * LN2,
                                op0=A.mult, op1=A.add)
        hrow2 = mats.tile([64, B], BF16, tag="hrow2")
        nc.vector.tensor_add(out=hrow2[0:64, :], in0=e1_t[0:64, :],
                             in1=lvh_t[0:64, :])

        # ---- prep G rows (i side) ----
        z2f = mats.tile([64, BL], F32, tag="z2f")
        nc.vector.tensor_mul(out=z2f[0:64, :], in0=z_t[0:64, :], in1=z_t[0:64, :])
        nz2_b = mats.tile([64, BL], BF16, tag="nz2_b")
        nc.vector.tensor_scalar(out=nz2_b[0:64, :], in0=z2f[0:64, :],
                                scalar1=-1.0, scalar2=None, op0=A.mult)
        z_b = mats.tile([64, BL], BF16, tag="z_b")
        nc.vector.tensor_copy(out=z_b[0:64, :], in_=z_t[0:64, :])
        ones_b = mats.tile([64, BL], BF16, tag="ones_b")
        nc.gpsimd.memset(ones_b, 1.0)

        # G_mega [3, 64*256]: per-(d,it) lhsT slices, d-major from [64,256] rows
        g_mega = mats.tile([3, D * BL], BF16, tag="g_mega")
        nc.sync.dma_start(out=g_mega[0:1, :], in_=nz2_b[0:64, :])
        nc.sync.dma_start(out=g_mega[1:2, :], in_=z_b[0:64, :])
        nc.sync.dma_start(out=g_mega[2:3, :], in_=ones_b[0:64, :])

        # A[i,d] accumulators, one per i-tile
        a_mat0 = mats.tile([128, D], F32, tag="a_mat0")
        a_mat1 = mats.tile([128, D], F32, tag="a_mat1")
        a_mat = [a_mat0, a_mat1]

        # ---- A hot loop: PE bilinear form -> ACT Exp -> DVE j-reduce ----
        with (
            tc.tile_pool(name="hp", bufs=2) as hp,
            tc.tile_pool(name="pa", bufs=2, space="PSUM") as pa,
            tc.tile_pool(name="ep", bufs=3) as ep,
        ):
            for c in range(NCH):
                hch = hp.tile([3, DCH * B], BF16, tag="hch")
                dsl = slice(c * DCH, (c + 1) * DCH)
                nc.sync.dma_start(out=hch[0:1, :], in_=hrow0[dsl, :])
                nc.sync.dma_start(out=hch[1:2, :], in_=hrow1[dsl, :])
                nc.sync.dma_start(out=hch[2:3, :], in_=hrow2[dsl, :])
                for dd in range(DCH):
                    d = c * DCH + dd
                    for it in range(2):
                        tp = pa.tile([128, B], F32, tag="tp")
                        lhs = g_mega[0:3, d * BL + it * 128: d * BL + (it + 1) * 128]
                        for jb in range(4):
                            nc.tensor.matmul(
                                tp[:, jb * 512:(jb + 1) * 512], lhsT=lhs,
                                rhs=hch[0:3, dd * B + jb * 512: dd * B + (jb + 1) * 512],
                                start=True, stop=True)
                        e_t = ep.tile([128, B], BF16, tag="e")
                        nc.scalar.activation(out=e_t, in_=tp, func=AF.Exp,
                                             bias=0.0, scale=1.0)
                        nc.vector.tensor_reduce(out=a_mat[it][:, d:d + 1],
                                                in_=e_t, axis=AX.X, op=A.add)

        # ---- S matmuls + logsumexp (pre-Ln part) ----
        nmxs, esums = [], []
        with (
            tc.tile_pool(name="psp", bufs=1, space="PSUM") as psp,
            tc.tile_pool(name="scr", bufs=2) as scr,
        ):
            for it in range(2):
                isl = slice(it * 128, (it + 1) * 128)
                sps = []
                for jb in range(4):
                    jsl = slice(jb * 512, (jb + 1) * 512)
                    sp = psp.tile([128, 512], F32, tag=f"sp{jb}")
                    nc.tensor.matmul(sp, lhsT=nz2_b[0:64, isl], rhs=hrow0[0:64, jsl],
                                     start=True, stop=False)
                    nc.tensor.matmul(sp, lhsT=z_b[0:64, isl], rhs=hrow1[0:64, jsl],
                                     start=False, stop=False)
                    nc.tensor.matmul(sp, lhsT=ones_b[0:64, 0:128], rhs=hrow2[0:64, jsl],
                                     start=False, stop=True)
                    sps.append(sp)
                mx4 = mats.tile([128, 4], F32, tag="mx4", bufs=2)
                for jb in range(4):
                    nc.vector.tensor_reduce(out=mx4[:, jb:jb + 1], in_=sps[jb],
                                            axis=AX.X, op=A.max)
                nmx = mats.tile([128, 1], F32, tag="nmx", bufs=2)
                nc.vector.tensor_reduce(out=nmx, in_=mx4, axis=AX.X, op=A.max,
                                        negate=True)
                es4 = mats.tile([128, 4], F32, tag="es4", bufs=2)
                for jb in range(4):
                    sc = scr.tile([128, 512], BF16, tag="sc")
                    nc.scalar.activation(out=sc, in_=sps[jb], func=AF.Exp,
                                         bias=nmx, scale=1.0,
                                         accum_out=es4[:, jb:jb + 1])
                esum = mats.tile([128, 1], F32, tag="esum", bufs=2)
                nc.vector.tensor_reduce(out=esum, in_=es4, axis=AX.X, op=A.add)
                nmxs.append(nmx)
                esums.append(esum)

        # ---- Ln epilogue + final scalars ----
        with tc.tile_pool(name="psm", bufs=2, space="PSUM") as psm:
            contrib = []
            for it in range(2):
                ln_a = mats.tile([128, D], F32, tag="ln_a", bufs=2)
                nc.scalar.activation(out=ln_a, in_=a_mat[it], func=AF.Ln,
                                     bias=0.0, scale=1.0)
                p_col = mats.tile([128, 1], F32, tag="p_col", bufs=2)
                nc.vector.tensor_reduce(out=p_col, in_=ln_a, axis=AX.X, op=A.add)
                lqz = mats.tile([128, 1], F32, tag="lqz", bufs=2)
                nc.scalar.activation(out=lqz, in_=esums[it], func=AF.Ln,
                                     bias=0.0, scale=1.0)
                mx = mats.tile([128, 1], F32, tag="mx", bufs=2)
                nc.vector.tensor_scalar(out=mx, in0=nmxs[it], scalar1=-1.0,
                                        scalar2=None, op0=A.mult)
                nc.vector.tensor_add(out=lqz, in0=lqz, in1=mx)
                ctr = mats.tile([128, 1], F32, tag="ctr", bufs=2)
                nc.vector.tensor_sub(out=ctr, in0=lqz, in1=p_col)
                contrib.append(ctr)

            fps = psm.tile([1, 2], F32, tag="fps")
            nc.tensor.matmul(fps[0:1, 0:1], lhsT=contrib[0], rhs=ones,
                             start=True, stop=False)
            nc.tensor.matmul(fps[0:1, 0:1], lhsT=contrib[1], rhs=ones,
                             start=False, stop=True)
            nc.tensor.matmul(fps[0:1, 1:2], lhsT=kss, rhs=ones,
                             start=True, stop=True)
            out_sb = mats.tile([1, 2], F32, tag="out_sb")
            nc.vector.tensor_copy(out=out_sb[0:1, :], in_=fps[0:1, :])
            nc.sync.dma_start(out=out_ext, in_=out_sb[0:1, :])


_NC_CACHE = {}


def _get_nc():
    if "nc" not in _NC_CACHE:
        nc = bacc.Bacc("TRN2", target_bir_lowering=False, debug=False,
                       num_devices=M)
        with tile.TileContext(nc) as tc:
            _body(tc)
        nc.compile()
        _NC_CACHE["nc"] = nc
    return _NC_CACHE["nc"]


def kernel(kl, z_mean, z_logvar, z_sampled, _trace=False, _tmpdir=None):
    kl = np.ascontiguousarray(kl, dtype=np.float32)
    z_mean = np.ascontiguousarray(z_mean, dtype=np.float32)
    z_logvar = np.ascontiguousarray(z_logvar, dtype=np.float32)
    z_sampled = np.ascontiguousarray(z_sampled, dtype=np.float32)
    nc = _get_nc()
    in_maps = []
    for c in range(M):
        sl = slice(c * BL, (c + 1) * BL)
        in_maps.append({
            "kl": np.ascontiguousarray(kl[sl]),
            "z_mean": z_mean,
            "z_logvar": z_logvar,
            "z_sampled": np.ascontiguousarray(z_sampled[sl]),
        })
    res = run_bass_kernel_spmd(nc, in_maps, list(range(M)), trace=_trace,
                               tmpdir=_tmpdir)
    t_sum = 0.0
    kl_sum = 0.0
    for c in range(M):
        o = res.results[c]["out"]
        t_sum += float(o[0, 0])
        kl_sum += float(o[0, 1])
    val = (BETA - 1.0) * (t_sum / B) + kl_sum
    out = np.float32(val)
    if _trace:
        return out, res
    return out


# revision 8
# speedup vs baseline: 2.4725x; 1.0480x over previous
"""BetaTCVAE loss kernel for 8 TRN2 NeuronCores (Bass/Tile).

Math
----
reference:  out = (BETA-1)*tc + sum(kl)
  lp[i,j,d] = -0.5*((z_i - m_j)^2 * exp(-lv_j) + lv_j + LOG2PI)   (per dim d)
  log_qz_product[i] = sum_d logsumexp_j lp[i,j,d]
  log_qz[i]         = logsumexp_j sum_d lp[i,j,d]
  tc = mean_i(log_qz - log_qz_product)

Decomposition used here (per core, rows i sharded 256/core):
  With s^2 = exp(-lv)/2 define the shifted exponent
      t[i,j,d] = -s2*z^2 + wm*z + e,
      wm = 2*s2*m,  e = -0.5*(wm*m + lv) - 0.5*ln2
  so that t = lp + (LOG2PI - ln2)/2 per dim.  Then
      A[i,d]   = sum_j exp(t[i,j,d])       ->  P[i] = sum_d ln A[i,d]
      S[i,j]   = sum_d t[i,j,d]            ->  lqz[i] = logsumexp_j S[i,j]
      contrib[i] = lqz[i] - P[i]           (shift constants cancel)
  tc = mean_i contrib.

Mapping to engines:
  * t for one latent dim d is a rank-3 bilinear form: one [3 x 128] lhsT
    (-z^2, z, 1 per row-i) against a [3 x 512] rhs (s2, wm, e per col-j)
    -> 4 matmuls fill a [128 x 2048] PSUM tile on the Tensor engine.
  * A[i,d] = one Activation instruction: Exp over the PSUM tile with
    accum_out giving the j-sum directly.  ACT is the only engine doing a
    full pass over the [256 x 2048 x 64] log-density volume.
  * S via three 64-contraction bf16 matmuls (same H rows), classic
    max-shifted logsumexp epilogue.
  * Final: out = (BETA-1)*(T_sum/B) + KL_sum  (host side).
"""

import math
import sys

import numpy as np

if "/opt/trn_rl_repo" not in sys.path:
    sys.path.insert(0, "/opt/trn_rl_repo")

import concourse.bacc as bacc
import concourse.tile as tile
from concourse import mybir
from concourse.bass_utils import run_bass_kernel_spmd
from concourse.masks import make_identity

B, D, M = 2048, 64, 8
BL = B // M          # 256 local rows
NJT = B // 128       # 16 natural j-tiles
DCH = 8              # latent dims per H chunk
NCH = D // DCH       # 8 chunks
F32 = mybir.dt.float32
BF16 = mybir.dt.bfloat16
LN2 = math.log(2.0)
BETA = 6.0

A = mybir.AluOpType
AF = mybir.ActivationFunctionType
AX = mybir.AxisListType


def _body(tc):
    nc = tc.nc
    kl_ext = nc.dram_tensor("kl", [BL, D], F32, kind="ExternalInput").ap()
    zm_ext = nc.dram_tensor("z_mean", [B, D], F32, kind="ExternalInput").ap()
    zlv_ext = nc.dram_tensor("z_logvar", [B, D], F32, kind="ExternalInput").ap()
    zs_ext = nc.dram_tensor("z_sampled", [BL, D], F32, kind="ExternalInput").ap()
    out_ext = nc.dram_tensor("out", [1, 2], F32, kind="ExternalOutput").ap()

    with (
        tc.tile_pool(name="cst", bufs=1) as cst,
        tc.tile_pool(name="mats", bufs=1) as mats,
        tc.tile_pool(name="ld", bufs=4) as ld,
    ):
        ident = cst.tile([128, 128], F32, tag="ident")
        make_identity(nc, ident)
        ones = cst.tile([128, 1], F32, tag="ones")
        nc.vector.memset(ones, 1.0)

        # ---- load + transpose z_mean, z_logvar -> [64, 2048] f32 ----
        m_t = mats.tile([64, B], F32, tag="m_t")
        lv_t = mats.tile([64, B], F32, tag="lv_t")
        z_t = mats.tile([64, BL], F32, tag="z_t")
        with tc.tile_pool(name="pst", bufs=4, space="PSUM") as pst:
            for t in range(NJT):
                nat = ld.tile([128, D], F32, tag="nat")
                nc.sync.dma_start(out=nat, in_=zm_ext[t * 128:(t + 1) * 128, :])
                ps = pst.tile([64, 128], F32, tag="tp")
                nc.tensor.transpose(ps, nat, ident)
                nc.vector.tensor_copy(out=m_t[0:64, t * 128:(t + 1) * 128], in_=ps)
            for t in range(NJT):
                nat = ld.tile([128, D], F32, tag="nat")
                nc.sync.dma_start(out=nat, in_=zlv_ext[t * 128:(t + 1) * 128, :])
                ps = pst.tile([64, 128], F32, tag="tp")
                nc.tensor.transpose(ps, nat, ident)
                nc.vector.tensor_copy(out=lv_t[0:64, t * 128:(t + 1) * 128], in_=ps)
            for t in range(2):
                nat = ld.tile([128, D], F32, tag="nat")
                nc.sync.dma_start(out=nat, in_=zs_ext[t * 128:(t + 1) * 128, :])
                ps = pst.tile([64, 128], F32, tag="tp")
                nc.tensor.transpose(ps, nat, ident)
                nc.vector.tensor_copy(out=z_t[0:64, t * 128:(t + 1) * 128], in_=ps)

        # ---- kl partial sum ----
        ks2 = mats.tile([128, 2], F32, tag="ks2")
        for t in range(2):
            klt = ld.tile([128, D], F32, tag="klt", bufs=2)
            nc.sync.dma_start(out=klt, in_=kl_ext[t * 128:(t + 1) * 128, :])
            nc.vector.tensor_reduce(out=ks2[:, t:t + 1], in_=klt, axis=AX.X, op=A.add)
        kss = mats.tile([128, 1], F32, tag="kss")
        nc.vector.tensor_reduce(out=kss, in_=ks2, axis=AX.X, op=A.add)

        # ---- prep H rows (j side, [64 d, 2048 j]) ----
        # s2 = exp(-lv)/2; wm = 2*s2*m; e = -0.5*(wm*m + lv) - ln2/2
        bias_l2 = cst.tile([128, 1], F32, tag="bias_l2")
        nc.gpsimd.memset(bias_l2, math.log(0.5))
        hrow0 = mats.tile([64, B], BF16, tag="hrow0")
        nc.scalar.activation(out=hrow0[0:64, :], in_=lv_t[0:64, :], func=AF.Exp,
                             bias=bias_l2[0:64, :], scale=-1.0)
        wm_t = mats.tile([64, B], F32, tag="wm_t")
        nc.vector.scalar_tensor_tensor(out=wm_t[0:64, :], in0=hrow0[0:64, :],
                                       scalar=2.0, in1=m_t[0:64, :],
                                       op0=A.mult, op1=A.mult)
        hrow1 = mats.tile([64, B], BF16, tag="hrow1")
        nc.vector.tensor_copy(out=hrow1[0:64, :], in_=wm_t[0:64, :])
        # e1 = (-0.5*wm)*m ; lvh = -0.5*lv - ln2/2 ; e = e1 + lvh
        e1_t = mats.tile([64, B], F32, tag="e1_t")
        nc.vector.scalar_tensor_tensor(out=e1_t[0:64, :], in0=wm_t[0:64, :],
                                       scalar=-0.5, in1=m_t[0:64, :],
                                       op0=A.mult, op1=A.mult)
        lvh_t = mats.tile([64, B], F32, tag="lvh_t")
        nc.vector.tensor_scalar(out=lvh_t[0:64, :], in0=lv_t[0:64, :],
                                scalar1=-0.5, scalar2=-0.5 * LN2,
                                op0=A.mult, op1=A.add)
        hrow2 = mats.tile([64, B], BF16, tag="hrow2")
        nc.vector.tensor_add(out=hrow2[0:64, :], in0=e1_t[0:64, :],
                             in1=lvh_t[0:64, :])

        # ---- prep G rows (i side) ----
        z2f = mats.tile([64, BL], F32, tag="z2f")
        nc.vector.tensor_mul(out=z2f[0:64, :], in0=z_t[0:64, :], in1=z_t[0:64, :])
        nz2_b = mats.tile([64, BL], BF16, tag="nz2_b")
        nc.vector.tensor_scalar(out=nz2_b[0:64, :], in0=z2f[0:64, :],
                                scalar1=-1.0, scalar2=None, op0=A.mult)
        z_b = mats.tile([64, BL], BF16, tag="z_b")
        nc.vector.tensor_copy(out=z_b[0:64, :], in_=z_t[0:64, :])
        ones_b = mats.tile([64, BL], BF16, tag="ones_b")
        nc.gpsimd.memset(ones_b, 1.0)

        # G_mega [3, 64*256]: per-(d,it) lhsT slices, d-major from [64,256] rows
        g_mega = mats.tile([3, D * BL], BF16, tag="g_mega")
        nc.sync.dma_start(out=g_mega[0:1, :], in_=nz2_b[0:64, :])
        nc.sync.dma_start(out=g_mega[1:2, :], in_=z_b[0:64, :])
        nc.sync.dma_start(out=g_mega[2:3, :], in_=ones_b[0:64, :])

        # A[i,d] accumulators, one per i-tile
        a_mat0 = mats.tile([128, D], F32, tag="a_mat0")
        a_mat1 = mats.tile([128, D], F32, tag="a_mat1")
        a_mat = [a_mat0, a_mat1]

        # ---- A hot loop: PE bilinear form -> ACT Exp -> DVE j-reduce ----
        with (
            tc.tile_pool(name="hp", bufs=2) as hp,
            tc.tile_pool(name="pa", bufs=2, space="PSUM") as pa,
            tc.tile_pool(name="ep", bufs=3) as ep,
        ):
            for c in range(NCH):
                hch = hp.tile([3, DCH * B], BF16, tag="hch")
                dsl = slice(c * DCH, (c + 1) * DCH)
                nc.sync.dma_start(out=hch[0:1, :], in_=hrow0[dsl, :])
                nc.sync.dma_start(out=hch[1:2, :], in_=hrow1[dsl, :])
                nc.sync.dma_start(out=hch[2:3, :], in_=hrow2[dsl, :])
                for dd in range(DCH):
                    d = c * DCH + dd
                    for it in range(2):
                        tp = pa.tile([128, B], F32, tag="tp")
                        lhs = g_mega[0:3, d * BL + it * 128: d * BL + (it + 1) * 128]
                        for jb in range(4):
                            nc.tensor.matmul(
                                tp[:, jb * 512:(jb + 1) * 512], lhsT=lhs,
                                rhs=hch[0:3, dd * B + jb * 512: dd * B + (jb + 1) * 512],
                                start=True, stop=True)
                        e_t = ep.tile([128, B], BF16, tag="e")
                        if dd == 0:
                            # ~1/8 of reduces ride the ACT accumulator to
                            # balance ACT vs DVE load
                            nc.scalar.activation(out=e_t, in_=tp, func=AF.Exp,
                                                 bias=0.0, scale=1.0,
                                                 accum_out=a_mat[it][:, d:d + 1])
                        else:
                            nc.scalar.activation(out=e_t, in_=tp, func=AF.Exp,
                                                 bias=0.0, scale=1.0)
                            nc.vector.tensor_reduce(out=a_mat[it][:, d:d + 1],
                                                    in_=e_t, axis=AX.X, op=A.add)

        # ---- S matmuls + logsumexp (pre-Ln part) ----
        nmxs, esums = [], []
        with (
            tc.tile_pool(name="psp", bufs=1, space="PSUM") as psp,
            tc.tile_pool(name="scr", bufs=2) as scr,
        ):
            for it in range(2):
                isl = slice(it * 128, (it + 1) * 128)
                sps = []
                for jb in range(4):
                    jsl = slice(jb * 512, (jb + 1) * 512)
                    sp = psp.tile([128, 512], F32, tag=f"sp{jb}")
                    nc.tensor.matmul(sp, lhsT=nz2_b[0:64, isl], rhs=hrow0[0:64, jsl],
                                     start=True, stop=False)
                    nc.tensor.matmul(sp, lhsT=z_b[0:64, isl], rhs=hrow1[0:64, jsl],
                                     start=False, stop=False)
                    nc.tensor.matmul(sp, lhsT=ones_b[0:64, 0:128], rhs=hrow2[0:64, jsl],
                                     start=False, stop=True)
                    sps.append(sp)
                mx4 = mats.tile([128, 4], F32, tag="mx4", bufs=2)
                for jb in range(4):
                    nc.vector.tensor_reduce(out=mx4[:, jb:jb + 1], in_=sps[jb],
                                            axis=AX.X, op=A.max)
                nmx = mats.tile([128, 1], F32, tag="nmx", bufs=2)
                nc.vector.tensor_reduce(out=nmx, in_=mx4, axis=AX.X, op=A.max,
                                        negate=True)
                es4 = mats.tile([128, 4], F32, tag="es4", bufs=2)
                for jb in range(4):
                    sc = scr.tile([128, 512], BF16, tag="sc")
                    nc.scalar.activation(out=sc, in_=sps[jb], func=AF.Exp,
                                         bias=nmx, scale=1.0,
                                         accum_out=es4[:, jb:jb + 1])
                esum = mats.tile([128, 1], F32, tag="esum", bufs=2)
                nc.vector.tensor_reduce(out=esum, in_=es4, axis=AX.X, op=A.add)
                nmxs.append(nmx)
                esums.append(esum)

        # ---- Ln epilogue + final scalars ----
        with tc.tile_pool(name="psm", bufs=2, space="PSUM") as psm:
            contrib = []
            for it in range(2):
                ln_a = mats.tile([128, D], F32, tag="ln_a", bufs=2)
                nc.scalar.activation(out=ln_a, in_=a_mat[it], func=AF.Ln,
                                     bias=0.0, scale=1.0)
                p_col = mats.tile([128, 1], F32, tag="p_col", bufs=2)
                nc.vector.tensor_reduce(out=p_col, in_=ln_a, axis=AX.X, op=A.add)
                lqz = mats.tile([128, 1], F32, tag="lqz", bufs=2)
                nc.scalar.activation(out=lqz, in_=esums[it], func=AF.Ln,
                                     bias=0.0, scale=1.0)
                mx = mats.tile([128, 1], F32, tag="mx", bufs=2)
                nc.vector.tensor_scalar(out=mx, in0=nmxs[it], scalar1=-1.0,
                                        scalar2=None, op0=A.mult)
                nc.vector.tensor_add(out=lqz, in0=lqz, in1=mx)
                ctr = mats.tile([128, 1], F32, tag="ctr", bufs=2)
                nc.vector.tensor_sub(out=ctr, in0=lqz, in1=p_col)
                contrib.append(ctr)

            fps = psm.tile([1, 2], F32, tag="fps")
            nc.tensor.matmul(fps[0:1, 0:1], lhsT=contrib[0], rhs=ones,
                             start=True, stop=False)
            nc.tensor.matmul(fps[0:1, 0:1], lhsT=contrib[1], rhs=ones,
                             start=False, stop=True)
            nc.tensor.matmul(fps[0:1, 1:2], lhsT=kss, rhs=ones,
                             start=True, stop=True)
            out_sb = mats.tile([1, 2], F32, tag="out_sb")
            nc.vector.tensor_copy(out=out_sb[0:1, :], in_=fps[0:1, :])
            nc.sync.dma_start(out=out_ext, in_=out_sb[0:1, :])


_NC_CACHE = {}


def _get_nc():
    if "nc" not in _NC_CACHE:
        nc = bacc.Bacc("TRN2", target_bir_lowering=False, debug=False,
                       num_devices=M)
        with tile.TileContext(nc) as tc:
            _body(tc)
        nc.compile()
        _NC_CACHE["nc"] = nc
    return _NC_CACHE["nc"]


def kernel(kl, z_mean, z_logvar, z_sampled, _trace=False, _tmpdir=None):
    kl = np.ascontiguousarray(kl, dtype=np.float32)
    z_mean = np.ascontiguousarray(z_mean, dtype=np.float32)
    z_logvar = np.ascontiguousarray(z_logvar, dtype=np.float32)
    z_sampled = np.ascontiguousarray(z_sampled, dtype=np.float32)
    nc = _get_nc()
    in_maps = []
    for c in range(M):
        sl = slice(c * BL, (c + 1) * BL)
        in_maps.append({
            "kl": np.ascontiguousarray(kl[sl]),
            "z_mean": z_mean,
            "z_logvar": z_logvar,
            "z_sampled": np.ascontiguousarray(z_sampled[sl]),
        })
    res = run_bass_kernel_spmd(nc, in_maps, list(range(M)), trace=_trace,
                               tmpdir=_tmpdir)
    t_sum = 0.0
    kl_sum = 0.0
    for c in range(M):
        o = res.results[c]["out"]
        t_sum += float(o[0, 0])
        kl_sum += float(o[0, 1])
    val = (BETA - 1.0) * (t_sum / B) + kl_sum
    out = np.float32(val)
    if _trace:
        return out, res
    return out
